# revision 37
# baseline (speedup 1.0000x reference)
"""MLA (multi-head latent attention) forward kernel for Trainium2, 8 NeuronCores.

Sharding v3: data-parallel over batch (B=2) x tensor-parallel over heads
(16 heads -> 4 groups of 4). Core c handles batch c//4, head-group c%4;
the host sums the 4 head-group o_proj partials per batch.

The x->qa projection (the single largest GEMM, ~46% of PE work when
replicated) is additionally TOKEN-sharded within each batch group: each
core computes qa for its own 512-token chunk, applies RMSNorm locally
(full feature rows are present under token sharding), and the scaled qa
is AllGather'ed in bf16 across the 4 cores of the batch group. The
AllGather on this platform is latency-bound (~75us, host-triggered
steps), so it is fired early (~45us in) and its latency is hidden under
the still-replicated x->ckv/rope + kv_b work, which the PE executes
while the gather flies. Everything downstream (q_b on gathered qa,
attention, o_proj) is head-sharded.

All matmuls are pure bf16 x bf16 (1 PE cycle/row at any free size,
fp32 PSUM accumulate), which also removes the f32r small-free-size 4x
penalty on causal-diagonal attention tiles. qn/qr/kn/v stay fully
SBUF-resident in bf16 (no DRAM roundtrip between phases).

Structure per core:
  A-qa: 12 qa chains on own chunk; sum-of-squares via ones-matmul
      chain software-pipelined behind them; rmsnorm scale in-place;
      bounce -> AllGather g_a (gpsimd DMA ring).
  ckv passes (all T, replicated): per 512-token pass: 4 ckv chains +
      rope chain + ssk chain; rmsnorm scale; kpe^T copies; kv_b
      (kn^T per head + v rows -> SBUF resident), software-pipelined one
      pass behind the chains so the PE never waits on the scale.
  Bq: per gathered 512-token pass: qn^T/qr^T -> SBUF resident.
  Attention per (head, 512-wide tq chunk) in S^T layout, causal:
      S^T = kn-tile.T @ qn + kpe-pad-tile.T @ qr-pair (rope zero-padded
      to K=128). P^T = exp(S^T*SCALE) in bf16 feeds AV directly.
      Column sums l via ones-matmul chain; O^T scaled by 1/l.
  o_proj: out[tq,:] = sum_h O^T[h].T @ WoT[h] -> DMA to DRAM.
"""

import sys

if "/opt/trn_rl_repo" not in sys.path:
    sys.path.insert(0, "/opt/trn_rl_repo")

import numpy as np

import concourse.bass as bass
import concourse.mybir as mybir
from concourse import bacc
from concourse.tile import TileContext

F32 = mybir.dt.float32
BF16 = mybir.dt.bfloat16

B, T, C = 2, 2048, 2048
H, HG = 16, 4  # total heads, heads per core
QL = 1536      # q lora
KVL = 512      # kv lora
ROPE = 64
NOPE = 128
QHD = NOPE + ROPE  # 192
VHD = 128
EPS = 1e-6
SCALE = QHD ** -0.5
MASK_VAL = -1e9  # added pre-scale; exp((s+MASK_VAL)*SCALE) == 0.0

NT = T // 128        # 16 t tiles
NC_TILES = C // 128  # 16 contraction tiles over C
NJQ = QL // 128      # 12
NJK = KVL // 128     # 4
TCH = 512            # tokens per core chunk (A-qa shard)
GROUPS = [[0, 1, 2, 3], [4, 5, 6, 7]]
GROUPS8 = [[0, 1, 2, 3, 4, 5, 6, 7]]


def make_causal_mask_T(nc, mask, mask_val):
    """Additive mask for S^T tiles: keep (0) where col >= row, else mask_val."""
    nc.gpsimd.memset(mask, 0.0)
    nc.gpsimd.affine_select(
        out=mask,
        in_=mask,
        compare_op=mybir.AluOpType.is_ge,
        fill=mask_val,
        base=0,
        pattern=[[1, mask.shape[1]]],
        channel_multiplier=-1,
    )


def build_program() -> bass.Bass:
    nc = bacc.Bacc(num_devices=8)

    # x packed per 512-token pass: [p, pass, ct, t'] = x[pass*512+t', ct*128+p]
    xT_pk = nc.dram_tensor("xT_pk", [128, 4, NC_TILES, TCH], BF16, kind="ExternalInput")
    xo_pk = nc.dram_tensor("xo_pk", [128, NC_TILES, TCH], BF16, kind="ExternalInput")
    wqa_pk = nc.dram_tensor("wqa_pk", [NJQ, 128, NC_TILES, 128], BF16, kind="ExternalInput")
    wkva_res = nc.dram_tensor("wkva_res", [128, NJK + 1, NC_TILES, 128], BF16, kind="ExternalInput")
    wqbT_n = nc.dram_tensor("wqbT_n", [QL, HG * NOPE], BF16, kind="ExternalInput")
    wqbT_r = nc.dram_tensor("wqbT_r", [QL, 2 * 128], BF16, kind="ExternalInput")
    wkvbT_n = nc.dram_tensor("wkvbT_n", [KVL, HG * NOPE], BF16, kind="ExternalInput")
    wkvbT_v = nc.dram_tensor("wkvbT_v", [KVL, HG * VHD], BF16, kind="ExternalInput")
    woT = nc.dram_tensor("woT", [128, HG * C], BF16, kind="ExternalInput")
    slot_base = nc.dram_tensor("slot_base", [1, 1], mybir.dt.uint32, kind="ExternalInput")
    out = nc.dram_tensor("out", [T, C], F32, kind="ExternalOutput")

    with TileContext(nc) as tc:
        with tc.tile_pool(name="dram", bufs=1, space="DRAM") as dram_pool:
            bounce_a = dram_pool.tile([128, NJQ * TCH], BF16)
            # 8-way gather into the Shared scratchpad: >4-core groups get
            # the shared-output collective path; each core reads its batch's
            # 4 slots via a per-core dynamic base index
            gath_a = dram_pool.tile([8, 128, NJQ * TCH], BF16, addr_space="Shared")
            _build_tiled(nc, tc, locals())
    nc.finalize()
    return nc


def _build_tiled(nc, tc, io):
    xT_pk, xo_pk = io["xT_pk"], io["xo_pk"]
    wqa_pk, wkva_res = io["wqa_pk"], io["wkva_res"]
    wqbT_n, wqbT_r = io["wqbT_n"], io["wqbT_r"]
    wkvbT_n, wkvbT_v, woT, out = io["wkvbT_n"], io["wkvbT_v"], io["woT"], io["out"]
    bounce_a, gath_a = io["bounce_a"], io["gath_a"]
    slot_base = io["slot_base"]

    from contextlib import ExitStack

    ctx = ExitStack()
    with ctx:
        # ---- small persistent constants ----
        const_pool = ctx.enter_context(tc.tile_pool(name="const", bufs=1))
        cmaskT = const_pool.tile([128, 128], F32)
        make_causal_mask_T(nc, cmaskT[:], mask_val=MASK_VAL)
        ones_stage = const_pool.tile([128, 128], F32)
        nc.vector.memset(ones_stage[:], 1.0)
        ones_bf = const_pool.tile([128, 128], BF16)
        nc.vector.tensor_copy(ones_bf[:], ones_stage[:])
        eps_t = const_pool.tile([128, 1], F32)
        nc.vector.memset(eps_t[:], EPS)
        zstage = const_pool.tile([128, 512], BF16)
        nc.vector.memset(zstage[:], 0.0)

        # ---- PE warmup (no data deps): hold the HAM un-throttled while
        # the first x / weight DMAs are in flight ----
        with tc.tile_pool(name="warm", bufs=1, space="PSUM") as wmpool:
            wm = wmpool.tile([128, 512], F32, tag="wm")
            # ~10us of warmup: bridges until the first qa chain's data
            # lands (~11us) so the PE never idles early and the p-state
            # ramp is never reset before the real work starts
            for i in range(48):
                nc.tensor.matmul(
                    wm[:], ones_bf[:], zstage[:],
                    start=(i == 0), stop=(i == 47), skip_group_check=True,
                )

        # ---- persistent SBUF-resident activations for attention ----
        kv_pool = ctx.enter_context(tc.tile_pool(name="kv", bufs=1))
        # zero-padded rope keys: kpe_e rows 0:64 = kpe (even heads),
        # kpe_o rows 64:128 = kpe (odd heads); other half stays zero
        kpe_e = kv_pool.tile([128, T], BF16)
        kpe_o = kv_pool.tile([128, T], BF16)
        nc.vector.memset(kpe_e[:], 0.0)
        nc.vector.memset(kpe_o[:], 0.0)
        kn_sb = kv_pool.tile([128, HG, T], BF16)   # k_nope^T per head
        v_sb = kv_pool.tile([128, NT, HG * VHD], BF16)  # v rows per t-tile
        qn_sb = kv_pool.tile([128, HG, T], BF16)   # q_nope^T per head
        qr_sb = kv_pool.tile([128, 2, T], BF16)    # q_rope^T per head pair
        # pass-0 x tile: loaded early (right behind the first two qa weight
        # tiles) so its transfer completes long before the collective flight
        xt0 = kv_pool.tile([128, NC_TILES, TCH], BF16)

        # resident B-phase weights (DMAs fired early in the A phase,
        # before any collective-waiting descriptor on the scalar ring)
        res_pool = ctx.enter_context(tc.tile_pool(name="res", bufs=1))
        wqn = res_pool.tile([128, NJQ, HG * NOPE], BF16)
        wqr = res_pool.tile([128, NJQ, 256], BF16)
        wn = res_pool.tile([128, NJK, HG * NOPE], BF16)
        wv = res_pool.tile([128, NJK, HG * VHD], BF16)

        def load_resident():
            # gpsimd software ring: executes ~35-50us late by design, which
            # moves these 4.4MB of transfers out of the oversubscribed
            # startup window (xo + wqa stream + xt0 + xt1 already demand
            # more than the per-core HBM budget); wn/wv are not needed
            # until the first kv_b pass (~105us), wqn/wqr until Bq (~200us)
            nc.gpsimd.dma_start(wqn[:], wqbT_n.rearrange("(j p) m -> p j m", p=128))
            nc.gpsimd.dma_start(wqr[:], wqbT_r.rearrange("(j p) m -> p j m", p=128))
            nc.gpsimd.dma_start(wn[:], wkvbT_n.rearrange("(k p) m -> p k m", p=128))
            nc.gpsimd.dma_start(wv[:], wkvbT_v.rearrange("(k p) m -> p k m", p=128))

        qas_pool = ctx.enter_context(tc.tile_pool(name="qas", bufs=2))
        pf_qa = {}
        # per-core gather base slot ((core//4)*4), loaded into a scalar-
        # engine register the same way partition_id() is
        _sb_reg = nc.scalar.alloc_register("slot_base_reg")
        nc.scalar.reg_load(_sb_reg, slot_base[0:1, 0:1])
        sb_val = nc.scalar.snap(_sb_reg, donate=True, min_val=0, max_val=4)

        def prefetch_qa(pa):
            t = qas_pool.tile([128, NJQ, TCH], BF16, tag="qas")
            nc.scalar.dma_start(t[:], gath_a[sb_val + pa])
            pf_qa[pa] = t

        sq_pool = ctx.enter_context(tc.tile_pool(name="p_sq", bufs=2))
        st_pool = ctx.enter_context(tc.tile_pool(name="p_st", bufs=1))

        def mk_ss(sstile, sq, sfirst, slast):
            def d():
                nc.tensor.matmul(
                    sstile[:], ones_bf[:], sq[:],
                    start=sfirst, stop=slast, skip_group_check=True,
                )
            return d

        # ================= A-qa: own 512-token chunk =================
        with (
            tc.tile_pool(name="p_xo", bufs=1) as xopool,
            tc.tile_pool(name="p_stage", bufs=1) as stagepool,
            tc.tile_pool(name="p_w", bufs=3) as wt_pool,
            tc.tile_pool(name="p_aps", bufs=2, space="PSUM") as apsum,
            tc.tile_pool(name="p_ssq", bufs=1, space="PSUM") as ssqpsum,
        ):
            wt_pf = {}

            def prefetch_wt(jt):
                t = wt_pool.tile([128, NC_TILES, 128], BF16, tag="wt")
                nc.sync.dma_start(t[:], wqa_pk[jt])
                wt_pf[jt] = t

            xo = xopool.tile([128, NC_TILES, TCH], BF16, tag="xo")
            nc.scalar.dma_start(xo[:], xo_pk[:])
            load_resident()

            qa_bf = stagepool.tile([128, NJQ, TCH], BF16)
            ssq = ssqpsum.tile([128, TCH], F32, tag="ssq")

            prefetch_wt(0)
            prefetch_wt(1)
            nc.sync.dma_start(xt0[:], xT_pk[:, 0])
            deferred = None
            for jt in range(NJQ):
                wt = wt_pf.pop(jt)
                if jt + 2 < NJQ:
                    prefetch_wt(jt + 2)
                ps = apsum.tile([128, TCH], F32, tag="achain")
                for ct in range(NC_TILES):
                    nc.tensor.matmul(
                        ps[:], wt[:, ct, :], xo[:, ct, :],
                        start=(ct == 0), stop=(ct == NC_TILES - 1),
                    )
                if deferred is not None:
                    deferred()
                sq = sq_pool.tile([128, TCH], BF16, tag="sq")
                nc.scalar.square(sq[:], ps[:])
                deferred = mk_ss(ssq, sq, jt == 0, jt == NJQ - 1)
                nc.vector.tensor_copy(qa_bf[:, jt, :], ps[:])
            deferred()  # final ssq matmul

            stdq = st_pool.tile([128, TCH], F32, tag="stdq")
            nc.scalar.activation(
                stdq[:], ssq[:],
                mybir.ActivationFunctionType.Sqrt,
                bias=eps_t[:], scale=1.0 / QL,
            )
            bcq = st_pool.tile([128, TCH], F32, tag="bcq")
            nc.vector.reciprocal(bcq[:], stdq[:])
            for jt in range(NJQ):
                nc.vector.tensor_mul(
                    out=qa_bf[:, jt, :], in0=qa_bf[:, jt, :], in1=bcq[:])
                if jt == NJQ // 2 - 1:
                    # first half ships while the second half is still scaling
                    nc.scalar.dma_start(
                        bounce_a[:, :NJQ // 2 * TCH], qa_bf[:, :NJQ // 2, :])
            nc.scalar.dma_start(
                bounce_a[:, NJQ // 2 * TCH:], qa_bf[:, NJQ // 2:, :])

        # ======= ckv passes over all T (replicated; hides the gather) =======
        # While the collective is in flight, host-triggered DMA descriptors
        # degrade to the ~8us cc-step cadence, so this phase pre-issues all
        # its data movement (resident ckv weights once, one xt descriptor
        # per pass, double-buffered) before/around the trigger.
        with (
            tc.tile_pool(name="p_x", bufs=2) as xpool,
            tc.tile_pool(name="p_ckvw", bufs=1) as ckvwpool,
            tc.tile_pool(name="p_ckv", bufs=2) as ckvpool,
            tc.tile_pool(name="p_aps2", bufs=2, space="PSUM") as apsum,
            tc.tile_pool(name="p_ssk", bufs=2, space="PSUM") as sskpsum,
            tc.tile_pool(name="p_kv", bufs=2, space="PSUM") as kvpsum,
        ):
            # ckv/rope chain weights are pass-invariant: resident, loaded
            # once from a host-packed contiguous layout (one descriptor)
            ckvw = ckvwpool.tile([128, NJK + 1, NC_TILES, 128], BF16)
            nc.sync.dma_start(ckvw[:], wkva_res[:])

            xts = {}

            def load_xt(pa):
                t = xpool.tile([128, NC_TILES, TCH], BF16, tag="xt")
                nc.sync.dma_start(t[:], xT_pk[:, pa])
                xts[pa] = t

            # emit the remaining xt descriptors BEFORE the collective
            # trigger: descriptors emitted after it crawl at the cc-step
            # cadence until the gather completes. xt1/xt2 get fresh buffer
            # slots (no reuse wait); only xt3 waits on pass-1 readers.
            for pa in range(1, 4):
                load_xt(pa)
            nc.gpsimd.collective_compute(
                "AllGather", mybir.AluOpType.bypass, replica_groups=GROUPS8,
                ins=[bounce_a.opt()], outs=[gath_a.opt()],
            )

            def make_bkv(pa, ckv_bf):
                tabs = pa * TCH

                def run():
                    for h in range(HG):
                        ps = kvpsum.tile([128, TCH], F32, tag="kvch")
                        for kj in range(NJK):
                            nc.tensor.matmul(
                                ps[:],
                                wn[:, kj, h * NOPE:(h + 1) * NOPE],
                                ckv_bf[:, kj, :],
                                start=(kj == 0),
                                stop=(kj == NJK - 1),
                            )
                        nc.vector.tensor_copy(
                            kn_sb[:, h, tabs:tabs + TCH], ps[:])
                    for tt in range(4):
                        ps = kvpsum.tile([128, TCH], F32, tag="kvch")
                        for kj in range(NJK):
                            nc.tensor.matmul(
                                ps[:],
                                ckv_bf[:, kj, tt * 128:(tt + 1) * 128],
                                wv[:, kj, :],
                                start=(kj == 0),
                                stop=(kj == NJK - 1),
                            )
                        nc.vector.tensor_copy(v_sb[:, 4 * pa + tt, :], ps[:])
                return run

            pend_bkv = None
            for pa in range(4):
                tabs = pa * TCH
                xt = xt0 if pa == 0 else xts.pop(pa)
                if pa == 3:
                    # all xt descriptors queued; the gather reads (gated on
                    # the collective) can go behind them now
                    prefetch_qa(0)
                    prefetch_qa(1)
                ckv_bf = ckvpool.tile([128, NJK, TCH], BF16, tag="ckvbf")
                ssk = sskpsum.tile([128, TCH], F32, tag="ssk")
                deferred = None
                for k in range(NJK + 1):
                    wcols = 128 if k < NJK else 64
                    ps = apsum.tile([128, TCH], F32, tag="achain")
                    for ct in range(NC_TILES):
                        nc.tensor.matmul(
                            ps[:wcols], ckvw[:, k, ct, :wcols], xt[:, ct, :],
                            start=(ct == 0), stop=(ct == NC_TILES - 1),
                        )
                    if deferred is not None:
                        deferred()
                        deferred = None
                    if k < NJK:
                        sq = sq_pool.tile([128, TCH], BF16, tag="sq")
                        nc.scalar.square(sq[:], ps[:])
                        deferred = mk_ss(ssk, sq, k == 0, k == NJK - 1)
                        nc.vector.tensor_copy(ckv_bf[:, k, :], ps[:])
                    else:
                        nc.vector.tensor_copy(
                            kpe_e[0:64, tabs:tabs + TCH], ps[:64])
                        nc.vector.tensor_copy(
                            kpe_o[64:128, tabs:tabs + TCH], ps[:64])
                assert deferred is None  # ssk last fired in rope iteration
                stdk = st_pool.tile([128, TCH], F32, tag="stdk")
                nc.scalar.activation(
                    stdk[:], ssk[:],
                    mybir.ActivationFunctionType.Sqrt,
                    bias=eps_t[:], scale=1.0 / KVL,
                )
                bck = st_pool.tile([128, TCH], F32, tag="bck")
                nc.vector.reciprocal(bck[:], stdk[:])
                for kj in range(NJK):
                    nc.vector.tensor_mul(
                        out=ckv_bf[:, kj, :], in0=ckv_bf[:, kj, :], in1=bck[:])
                # previous pass's kv_b runs now: the PE fills the window
                # while this pass's scale completes on ACT/DVE
                if pend_bkv is not None:
                    pend_bkv()
                pend_bkv = make_bkv(pa, ckv_bf)
            pend_bkv()

        # ================= Bq: qn/qr from gathered qa =================
        with tc.tile_pool(name="p_bq", bufs=2, space="PSUM") as bqpsum:
            for pa in range(4):
                tabs = pa * TCH
                qa_sb = pf_qa.pop(pa)
                if pa + 2 < 4:
                    prefetch_qa(pa + 2)
                for g in range(6):
                    ps = bqpsum.tile([128, TCH], F32, tag="bq")
                    for jt in range(NJQ):
                        if g < HG:
                            lhs = wqn[:, jt, g * NOPE:(g + 1) * NOPE]
                        else:
                            lhs = wqr[:, jt, (g - HG) * 128:(g - HG + 1) * 128]
                        nc.tensor.matmul(
                            ps[:],
                            lhs,
                            qa_sb[:, jt, :],
                            start=(jt == 0),
                            stop=(jt == NJQ - 1),
                        )
                    if g < HG:
                        nc.vector.tensor_copy(qn_sb[:, g, tabs:tabs + TCH], ps[:])
                    else:
                        nc.vector.tensor_copy(
                            qr_sb[:, g - HG, tabs:tabs + TCH], ps[:])

        # ================= Attention + o_proj (S^T layout) =================
        with (
            tc.tile_pool(name="at_wo", bufs=1) as wopool,
            tc.tile_pool(name="at_pt", bufs=4) as ptpool,
            tc.tile_pool(name="at_st", bufs=2) as astpool,
            tc.tile_pool(name="at_ot", bufs=2) as otpool,
            tc.tile_pool(name="at_ob", bufs=4) as obpool,
            tc.tile_pool(name="at_sps", bufs=2, space="PSUM") as spsum,
            tc.tile_pool(name="at_avps", bufs=2, space="PSUM") as avpsum,
            tc.tile_pool(name="at_lps", bufs=2, space="PSUM") as lpsum,
            tc.tile_pool(name="at_ops", bufs=2, space="PSUM") as opsum,
        ):
            wo_sb = wopool.tile([128, HG, C], BF16)   # o_proj weights
            nc.sync.dma_start(wo_sb[:], woT.rearrange("p (h c) -> p h c", c=C))
            for c in (3, 2, 1, 0):  # 512-wide tq chunks, dense first
                q0 = c * 512
                ntk = 4 * c + 4
                ot_sb = otpool.tile([128, HG, 512], BF16, tag="ot")
                for h in range(HG):
                    qn_t = qn_sb[:, h, q0:q0 + 512]
                    qr_t = qr_sb[:, h // 2, q0:q0 + 512]
                    kpe_h = kpe_e if h % 2 == 0 else kpe_o
                    av = avpsum.tile([128, 512], F32, tag="av")
                    lch = lpsum.tile([128, 512], F32, tag="l")

                    pts, offs = [], []

                    def s_stage(j, c=c, h=h, qn_t=qn_t, qr_t=qr_t,
                                kpe_h=kpe_h, pts=pts, offs=offs):
                        off = max(0, (j - 4 * c) * 128)
                        ps = spsum.tile([128, 512], F32, tag="schain")
                        nc.tensor.matmul(
                            ps[:, off:512],
                            kn_sb[:, h, j * 128:(j + 1) * 128],
                            qn_t[:, off:512],
                            start=True,
                            stop=False,
                        )
                        nc.tensor.matmul(
                            ps[:, off:512],
                            kpe_h[:, j * 128:(j + 1) * 128],
                            qr_t[:, off:512],
                            start=False,
                            stop=True,
                        )
                        if j >= 4 * c:
                            nc.vector.tensor_add(
                                out=ps[:, off:off + 128],
                                in0=ps[:, off:off + 128],
                                in1=cmaskT[:],
                            )
                        pt = ptpool.tile([128, 512], BF16, tag="pt")
                        nc.scalar.activation(
                            pt[:, off:512],
                            ps[:, off:512],
                            mybir.ActivationFunctionType.Exp,
                            scale=SCALE,
                        )
                        pts.append(pt)
                        offs.append(off)

                    def av_stage(j, h=h, av=av, lch=lch, pts=pts, offs=offs,
                                 ntk=ntk):
                        off = offs[j]
                        nc.tensor.matmul(
                            lch[:, off:512],
                            ones_bf[:],
                            pts[j][:, off:512],
                            start=(j == 0),
                            stop=(j == ntk - 1),
                            skip_group_check=True,
                        )
                        nc.tensor.matmul(
                            av[:, off:512],
                            v_sb[:, j, h * VHD:(h + 1) * VHD],
                            pts[j][:, off:512],
                            start=(j == 0),
                            stop=(j == ntk - 1),
                            skip_group_check=True,
                        )

                    for j0 in range(min(2, ntk)):
                        s_stage(j0)
                    for j in range(ntk):
                        if j + 2 < ntk:
                            s_stage(j + 2)
                        av_stage(j)

                    linv = astpool.tile([128, 512], F32, tag="linv")
                    nc.vector.reciprocal(linv[:], lch[:])
                    nc.vector.tensor_mul(
                        out=ot_sb[:, h, :], in0=av[:], in1=linv[:]
                    )

                # o_proj for these 512 rows
                for s in range(4):
                    trow = q0 + s * 128
                    for cn in range(C // 512):
                        ps = opsum.tile([128, 512], F32, tag="oproj")
                        for h in range(HG):
                            nc.tensor.matmul(
                                ps[:],
                                ot_sb[:, h, s * 128:(s + 1) * 128],
                                wo_sb[:, h, cn * 512:(cn + 1) * 512],
                                start=(h == 0),
                                stop=(h == HG - 1),
                            )
                        osb = obpool.tile([128, 512], F32, tag="osb")
                        nc.vector.tensor_copy(osb[:], ps[:])
                        nc.sync.dma_start(
                            out[trow:trow + 128, cn * 512:(cn + 1) * 512], osb[:]
                        )


_PROGRAM_CACHE = {}


def _get_program():
    if "nc" not in _PROGRAM_CACHE:
        _PROGRAM_CACHE["nc"] = build_program()
    return _PROGRAM_CACHE["nc"]


def _shard_weights(Wqa, gqa, Wqb, Wkva, gkva, Wkvb, Wo, hg):
    import ml_dtypes
    bf16 = ml_dtypes.bfloat16
    h0 = hg * HG
    Wqb_s = (Wqb * gqa[None, :]).reshape(H, QHD, QL)
    Wn = Wqb_s[h0:h0 + HG, :NOPE, :]                    # [4,128,QL]
    Wr = Wqb_s[h0:h0 + HG, NOPE:, :]                    # [4,64,QL]
    wqbT_n = np.ascontiguousarray(Wn.reshape(HG * NOPE, QL).T)
    wqbT_r = np.ascontiguousarray(
        Wr.reshape(2, 128, QL).transpose(2, 0, 1).reshape(QL, 256))
    Wkvb_s = (Wkvb * gkva[None, :]).reshape(H, NOPE + VHD, KVL)
    wkvbT_n = np.ascontiguousarray(
        Wkvb_s[h0:h0 + HG, :NOPE, :].reshape(HG * NOPE, KVL).T)
    wkvbT_v = np.ascontiguousarray(
        Wkvb_s[h0:h0 + HG, NOPE:, :].reshape(HG * VHD, KVL).T)
    # woT packed [128, HG*C]: partition = dv, free = (h, c)
    WoT = Wo[:, h0 * VHD:(h0 + HG) * VHD].T             # [512, C]
    woT = np.ascontiguousarray(
        WoT.reshape(HG, VHD, C).transpose(1, 0, 2).reshape(VHD, HG * C))
    return {
        "wqbT_n": wqbT_n.astype(bf16),
        "wqbT_r": wqbT_r.astype(bf16),
        "wkvbT_n": wkvbT_n.astype(bf16),
        "wkvbT_v": wkvbT_v.astype(bf16),
        "woT": woT.astype(bf16),
    }


def kernel(x, Wqa, gqa, Wqb, Wkva, gkva, Wkvb, Wo):
    from concourse.bass_utils import run_bass_kernel_spmd

    x = np.asarray(x, np.float32)
    args = [np.asarray(a, np.float32) for a in (Wqa, gqa, Wqb, Wkva, gkva, Wkvb, Wo)]
    Wqa, gqa, Wqb, Wkva, gkva, Wkvb, Wo = args

    nc = _get_program()
    # pack A weights so each [128,16,128] SBUF tile is one contiguous DMA:
    # pk[jt, p, ct, col] = W[jt*128+col, ct*128+p]
    import ml_dtypes
    bf16 = ml_dtypes.bfloat16
    wqa_pk_bf = np.ascontiguousarray(
        Wqa.reshape(NJQ, 128, NC_TILES, 128).transpose(0, 3, 2, 1)).astype(bf16)
    wkva_pk = Wkva[:KVL].reshape(NJK, 128, NC_TILES, 128).transpose(0, 3, 2, 1)
    wrope_pk = Wkva[KVL:].reshape(ROPE, NC_TILES, 128).transpose(2, 1, 0)
    wkva_res = np.zeros((128, NJK + 1, NC_TILES, 128), np.float32)
    wkva_res[:, :NJK] = wkva_pk.transpose(1, 0, 2, 3)
    wkva_res[:, NJK, :, :ROPE] = wrope_pk
    wkva_res = np.ascontiguousarray(wkva_res).astype(bf16)
    shard_cache = [
        _shard_weights(Wqa, gqa, Wqb, Wkva, gkva, Wkvb, Wo, hg) for hg in range(4)
    ]
    xT_pk = [np.ascontiguousarray(
        x[b].reshape(4, TCH, NC_TILES, 128).transpose(3, 0, 2, 1)).astype(bf16)
        for b in range(B)]

    in_maps = []
    for core in range(8):
        b, r = core // 4, core % 4
        m = {"xT_pk": xT_pk[b],
             "xo_pk": np.ascontiguousarray(xT_pk[b][:, r]),
             "slot_base": np.array([[b * 4]], np.uint32),
             "wqa_pk": wqa_pk_bf, "wkva_res": wkva_res}
        m.update(shard_cache[r])
        in_maps.append(m)

    res = run_bass_kernel_spmd(nc, in_maps, core_ids=list(range(8)))
    out = np.zeros((B, T, C), np.float32)
    for core in range(8):
        out[core // 4] += res.results[core]["out"]
    return out
